# revision 43
# baseline (speedup 1.0000x reference)
"""Trainium2 Bass kernel for nn_CVQNN: batched 5-layer CV quantum circuit.

Math: the 5 per-layer 15x15 unitaries depend only on 35 scalars. We fuse
them on the host (complex128) into one matrix U with psi_out = psi_in @ U.T,
then express the complex matmul as a real (B,30) @ (30,30) matmul on the
interleaved-float32 view of the complex64 batch.

Precision: batch and W are cast to float16 (PSUM accumulation stays fp32).
psi amplitudes and |U| entries are all <= ~1.5, so fp16's 11-bit mantissa
gives ~8e-5 relative error vs the reference — and halves all DMA traffic,
which is the roofline here. Host casts the interleaved view to f16; the
f16 output is upcast back on the host.

Device (per core, pure data parallel over 8 cores, 131072 rows each):
  - DRAM layout: x viewed as (128, 30720) f16 — partition p holds rows
    [1024p, 1024p+1024) contiguously, so slab DMAs are fully contiguous.
  - Per 512-row tile (128 partitions x 120 elems = 4 rows/partition):
      PE transpose (128,120) -> psiT (120,128) in PSUM   [f16, 1 cyc/row]
      copy psiT -> SBUF (batched 4 tiles per copy, DVE)
      PE matmul  psiT.T @ W_blk -> (128,120) in PSUM     [f16, 1 cyc/row]
      copy -> SBUF out slab (batched, DVE/ACT balanced ~5:1)
    where W_blk = block_diag(M,M,M,M), M the 30x30 real form of U.T.
  - Software-pipelined emission (transposes of group g+1 before matmuls of
    group g) so the PE never stalls on the PSUM->SBUF copy latency.
  - Input loads on the SP HWDGE ring; output stores alternate between the
    ACT and SP rings so consecutive stores overlap their setup latency and
    the store backlog drains on both rings once inputs finish.
"""

import numpy as np

CUTOFF = 15
N_LAYERS = 5
N_CORES = 8
BATCH = 1048576
ROWS_PER_CORE = BATCH // N_CORES          # 131072
F_TOTAL = ROWS_PER_CORE * 2 * CUTOFF // 128   # 30720 f32 per partition
SLAB_F = 3840                              # elems per partition per slab
OUT_SPLIT = 1                              # output stores per slab
N_SLABS = F_TOTAL // SLAB_F                # 4
TILE_F = 120                               # 4 rows x 30 f32
TILES_PER_GROUP = 4                        # batched PSUM->SBUF copies
GROUP_F = TILE_F * TILES_PER_GROUP         # 480
N_GROUPS = SLAB_F // GROUP_F               # 16


# ----------------------------------------------------------------------------
# Host math: fused unitary (complex128 recurrences, thewalrus conventions)
# ----------------------------------------------------------------------------

def _squeeze_mat(r, theta):
    c = CUTOFF
    sq = np.sqrt(np.arange(c, dtype=np.float64))
    T = np.exp(1j * theta) * np.tanh(r)
    Tc = np.conj(T)
    sech = 1.0 / np.cosh(r)
    S = np.zeros((c, c), dtype=np.complex128)
    S[0, 0] = np.sqrt(sech)
    for m in range(2, c, 2):
        S[m, 0] = -(sq[m - 1] / sq[m]) * T * S[m - 2, 0]
    for n in range(1, c):
        for m in range(c):
            if (m + n) % 2 == 0:
                val = 0.0 + 0.0j
                if n >= 2:
                    val = (sq[n - 1] / sq[n]) * Tc * S[m, n - 2]
                if m >= 1:
                    val = val + (sq[m] / sq[n]) * sech * S[m - 1, n - 1]
                S[m, n] = val
    return S


def _disp_mat(r, phi):
    c = CUTOFF
    sq = np.sqrt(np.arange(c, dtype=np.float64))
    alpha = r * np.exp(1j * phi)
    malphac = -r * np.exp(-1j * phi)
    D = np.zeros((c, c), dtype=np.complex128)
    D[0, 0] = np.exp(-0.5 * r * r)
    for m in range(1, c):
        D[m, 0] = (alpha / sq[m]) * D[m - 1, 0]
    for n in range(1, c):
        D[0, n] = (malphac / sq[n]) * D[0, n - 1]
        for m in range(1, c):
            D[m, n] = (malphac / sq[n]) * D[m, n - 1] + (sq[m] / sq[n]) * D[m - 1, n - 1]
    return D


def _layer_u(th1, sr, sth, th2, dr, dphi, kap):
    n = np.arange(CUTOFF, dtype=np.float64)
    p1 = np.exp(1j * th1 * n)
    p2 = np.exp(1j * th2 * n)
    kv = np.exp(1j * kap * n * n)
    S = _squeeze_mat(sr, sth)
    D = _disp_mat(dr, dphi)
    return (kv[:, None] * D) @ (p2[:, None] * S * p1[None, :])


def _total_unitary(theta1, sq_r, sq_theta, theta2, dis_r, dis_phi, kappa):
    U = np.eye(CUTOFF, dtype=np.complex128)
    for i in range(N_LAYERS):
        Ui = _layer_u(
            float(theta1[i]), float(sq_r[i]), float(sq_theta[i]), float(theta2[i]),
            float(dis_r[i]), float(dis_phi[i]), float(kappa[i]),
        )
        U = Ui @ U
    return U


def _real_matrix(U):
    """30x30 real M: x_interleaved @ M == interleaved(psi @ U.T)."""
    G = U.T
    M = np.zeros((2 * CUTOFF, 2 * CUTOFF), dtype=np.float64)
    M[0::2, 0::2] = G.real
    M[1::2, 0::2] = -G.imag
    M[0::2, 1::2] = G.imag
    M[1::2, 1::2] = G.real
    return M.astype(np.float32)


def _block_diag4(M):
    W = np.zeros((4 * 2 * CUTOFF, 4 * 2 * CUTOFF), dtype=np.float32)
    for r in range(4):
        W[r * 30:(r + 1) * 30, r * 30:(r + 1) * 30] = M
    return W


# ----------------------------------------------------------------------------
# Device program (built once, cached)
# ----------------------------------------------------------------------------

_NC_CACHE = {}


def _build_program(f_total=F_TOTAL):
    if f_total in _NC_CACHE:
        return _NC_CACHE[f_total]

    from contextlib import ExitStack

    import concourse.bass as bass
    import concourse.tile as tile
    from concourse import bacc, mybir

    f32 = mybir.dt.float32
    f16 = mybir.dt.float16
    slab_f = min(SLAB_F, f_total)
    if f_total > 4 * slab_f:
        # taper: small slabs at the edges for faster pipeline fill/drain
        half = slab_f // 2
        slab_sizes = [half, half]
        rem = f_total - 2 * slab_f
        slab_sizes += [slab_f] * (rem // slab_f)
        slab_sizes += [half, half]
    else:
        slab_sizes = [slab_f] * (f_total // slab_f)
    assert sum(slab_sizes) == f_total

    nc = bacc.Bacc(
        "TRN2",
        target_bir_lowering=False,
        debug=False,
        enable_asserts=False,
        num_devices=N_CORES,
    )

    x = nc.dram_tensor("x", [128, f_total], f16, kind="ExternalInput").ap()
    w = nc.dram_tensor("w", [TILE_F, TILE_F], f16, kind="ExternalInput").ap()
    idn = nc.dram_tensor("idn", [128, 128], f16, kind="ExternalInput").ap()
    y = nc.dram_tensor("y", [128, f_total], f16, kind="ExternalOutput").ap()

    with tile.TileContext(nc) as tc, ExitStack() as ctx:
        const = ctx.enter_context(tc.tile_pool(name="const", bufs=1))
        in_pool = ctx.enter_context(tc.tile_pool(name="xin", bufs=6))
        out_pool = ctx.enter_context(tc.tile_pool(name="yout", bufs=4))
        sbT_pool = ctx.enter_context(tc.tile_pool(name="sbT", bufs=4))
        psT_pool = ctx.enter_context(tc.tile_pool(name="psT", bufs=4, space="PSUM"))
        psO_pool = ctx.enter_context(tc.tile_pool(name="psO", bufs=4, space="PSUM"))

        # first slab load goes out before anything else
        xin0 = in_pool.tile([128, slab_sizes[0]], f16, tag="xin")
        nc.sync.dma_start(xin0[:], x[:, bass.ds(0, slab_sizes[0])])
        # consts ride the (initially idle) ACT ring so they don't delay
        # the input stream on the SP ring
        ident = const.tile([128, 128], f16)
        nc.scalar.dma_start(ident[:], idn[:])
        wblk = const.tile([TILE_F, TILE_F], f16)
        nc.scalar.dma_start(wblk[:], w[:])

        def mm_stage(ent, gidx):
            """Matmuls + PSUM->SBUF copies for a transposed group."""
            sbT, yout_t, g, s_off, s_f, s_idx = ent
            psO = psO_pool.tile([128, GROUP_F], f32)
            for k in range(TILES_PER_GROUP):
                nc.tensor.matmul(
                    psO[:, TILE_F * k:TILE_F * (k + 1)],
                    sbT[:, 128 * k:128 * (k + 1)],
                    wblk[:],
                    start=True,
                    stop=True,
                )
            if gidx % 5 == 0:
                nc.vector.tensor_copy(yout_t[:, bass.ts(g, GROUP_F)], psO[:])
            else:
                nc.scalar.copy(yout_t[:, bass.ts(g, GROUP_F)], psO[:])
            half = max(s_f // OUT_SPLIT, GROUP_F)
            if ((g + 1) * GROUP_F) % half == 0:
                # stores alternate between the two HWDGE rings: keeps setup
                # latencies of consecutive stores overlapped, and the tail
                # drains on both rings once inputs are done
                h = (g + 1) * GROUP_F // half - 1
                eng = nc.scalar if (s_idx * OUT_SPLIT + h) % 2 == 0 else nc.sync
                eng.dma_start(y[:, bass.ds(s_off + h * half, half)],
                              yout_t[:, bass.ds(h * half, half)])

        from collections import deque
        pending = deque()
        SKEW = 2
        gidx = 0
        off = 0
        for s, s_f in enumerate(slab_sizes):
            if s == 0:
                xin = xin0
            else:
                xin = in_pool.tile([128, s_f], f16, tag="xin")
                nc.sync.dma_start(xin[:], x[:, bass.ds(off, s_f)])
            yout = out_pool.tile([128, s_f], f16, tag="yout")
            n_tiles = s_f // TILE_F

            for g in range(s_f // GROUP_F):
                psT = psT_pool.tile([128, 512], f16)
                for k in range(TILES_PER_GROUP):
                    j = TILES_PER_GROUP * g + k
                    # 128-wide stationary (overreads into tile j+1) so FWL
                    # kicks in; extra out partitions 120..127 are ignored.
                    w_cols = 128 if j + 1 < n_tiles else TILE_F
                    nc.tensor.transpose(
                        psT[:w_cols, 128 * k:128 * (k + 1)],
                        xin[:, TILE_F * j:TILE_F * j + w_cols],
                        ident[:],
                    )
                sbT = sbT_pool.tile([TILE_F, 512], f16)
                nc.vector.tensor_copy(sbT[:], psT[:TILE_F, :])

                # software skew: PE has later groups' transposes to chew on
                # while earlier groups' copy1 completes
                if len(pending) >= SKEW:
                    mm_stage(pending.popleft(), gidx)
                    gidx += 1
                pending.append((sbT, yout, g, off, s_f, s))

            off += s_f

        while pending:
            mm_stage(pending.popleft(), gidx)
            gidx += 1

    nc.compile()
    _NC_CACHE[f_total] = nc
    return nc


# ----------------------------------------------------------------------------
# Entry point
# ----------------------------------------------------------------------------

def kernel(psi0, theta1, sq_r, sq_theta, theta2, dis_r, dis_phi, kappa):
    from concourse.bass_utils import run_bass_kernel_spmd

    nc = _build_program()

    U = _total_unitary(theta1, sq_r, sq_theta, theta2, dis_r, dis_phi, kappa)
    W = _block_diag4(_real_matrix(U)).astype(np.float16)

    psi0 = np.ascontiguousarray(psi0)
    assert psi0.dtype == np.complex64 and psi0.shape == (BATCH, CUTOFF)
    xf = psi0.view(np.float32).reshape(N_CORES, 128, F_TOTAL).astype(np.float16)

    eye = np.eye(128, dtype=np.float16)
    in_maps = [{"x": xf[c], "w": W, "idn": eye} for c in range(N_CORES)]
    res = run_bass_kernel_spmd(nc, in_maps, core_ids=list(range(N_CORES)))

    out = np.empty((BATCH, 2 * CUTOFF), dtype=np.float32)
    for c in range(N_CORES):
        out[c * ROWS_PER_CORE:(c + 1) * ROWS_PER_CORE] = (
            res.results[c]["y"].reshape(ROWS_PER_CORE, 2 * CUTOFF).astype(np.float32)
        )
    return out.view(np.complex64).reshape(BATCH, CUTOFF)
